# revision 1
# baseline (speedup 1.0000x reference)
"""Trainium2 Bass kernel for nn_KC_Avg_Embedding (multi-hot averaged embedding).

Computes, for multi-hot indicator vectors x[b,s,:] over a vocabulary of 1024:
    out[b,s,:] = (x[b,s,:] @ E) / max(sum(x[b,s,:]), 1)

Strategy (data-parallel over 8 NeuronCores, batch-sharded):
  - Each core gets rows = (B/8)*S = 3200 rows of x [3200, 1024] fp32 plus the
    full embedding matrix E [1024, 128] fp32.
  - x is DMA'd in with an fp32->bf16 cast (SWDGE); x is 0/1 so bf16 is exact.
  - Each [128, 128] block of x is transposed on the TensorEngine (via
    identity-matmul transpose) so the vocab dim lands on partitions.
  - E is split on-chip into bf16 hi + lo parts (E = hi + lo to ~2^-17 rel) and
    extended with a ones column; 16 accumulating bf16 matmuls per row-tile
    produce [128 rows, 129] in PSUM = [x@E | row_count] with fp32 accumulation.
  - Epilogue: out = psum[:, :128] * (1 / max(psum[:, 128], 1)).
"""

import sys
from contextlib import ExitStack

import numpy as np

for _p in ("/opt/trn_rl_repo",):
    if _p not in sys.path:
        sys.path.insert(0, _p)

import concourse.bass as bass
import concourse.mybir as mybir
import concourse.tile as tile
from concourse.masks import make_identity

from concourse.vector_clock import ScopedClock


class _SplitDrainTC(tile.TileContext):
    """TileContext whose kernel-tail drain splits its semaphore waits across
    single-wait carrier nops — this walrus build enforces a small
    per-instruction sync-wait limit that the stock all-lane drain exceeds."""

    def _drain_and_barrier(self, tick_clock, wait_clock):
        drain_inst = self.nc.sync.drain()
        wait_clock.add_sem_waits(
            drain_inst.ins, ScopedClock({None: tick_clock.global_clock})
        )
        si = drain_inst.ins.sync_info
        if si is not None and si.on_wait is not None and len(si.on_wait) > 1:
            waits = list(si.on_wait)
            del si.on_wait[1:]
            for w in waits[1:]:
                nop = self.nc.sync.nop(nofuse=True, hint="drain_wait_split")
                nsi = nop.ins.sync_info
                if nsi is None:
                    nop.ins.sync_info = mybir.SyncInfo(on_update=[], on_wait=[w])
                else:
                    nsi.on_wait.append(w)
        self.nc.all_engine_barrier()
        assert self.sems is not None
        popped = self.nc._tile_sem_poison_stack.pop()
        assert popped is self._sem_poison
        self.nc.clear_and_free_semaphores(list(self.sems.allocated().values()))
        self.nc.all_engine_barrier()


B, S, V, D = 128, 200, 1024, 128
NCORES = 8
P = 128
PER_CORE_B = B // NCORES          # 16
ROWS = PER_CORE_B * S             # 3200 rows per core
NCH = V // P                      # 8 vocab chunks
NE = D + 1                        # 128 emb cols + 1 count col


def build_kernel(rows=ROWS, group=5):
    """Build the per-core Bass program. `rows` must be a multiple of 128*group."""
    rt = rows // P                 # row tiles
    assert rt % group == 0
    ng = rt // group               # DMA groups

    nc = bass.Bass()
    x = nc.declare_dram_parameter("x", [rows, V], mybir.dt.float32, isOutput=False)
    emb = nc.declare_dram_parameter("emb", [V, D], mybir.dt.float32, isOutput=False)
    y = nc.declare_dram_parameter("y", [rows, D], mybir.dt.float32, isOutput=True)

    bf16 = mybir.dt.bfloat16
    f32 = mybir.dt.float32

    with _SplitDrainTC(nc) as tc, ExitStack() as ctx:
        const = ctx.enter_context(tc.tile_pool(name="const", bufs=1))
        # one slot per group: avoids slot-reuse waits that push instructions
        # over walrus' one-sync-wait-per-instruction codegen limit
        xb_pool = ctx.enter_context(tc.tile_pool(name="xb", bufs=ng))
        xt_pool = ctx.enter_context(tc.tile_pool(name="xt", bufs=4))
        out_pool = ctx.enter_context(tc.tile_pool(name="out", bufs=ng))
        small = ctx.enter_context(tc.tile_pool(name="small", bufs=4))
        psum_t = ctx.enter_context(tc.tile_pool(name="psum_t", bufs=2, space="PSUM"))
        psum_o = ctx.enter_context(tc.tile_pool(name="psum_o", bufs=2, space="PSUM"))

        # identity for TensorE transposes
        ident = const.tile([P, P], bf16)
        make_identity(nc, ident)

        # E -> bf16 hi/lo split, chunked [p, chunk, d], plus ones/zeros count col
        e_f32 = const.tile([P, NCH, D], f32)
        nc.sync.dma_start(e_f32[:], emb.rearrange("(c p) d -> p c d", p=P))
        rhs_hi = const.tile([P, NCH, NE], bf16)
        rhs_lo = const.tile([P, NCH, NE], bf16)
        e_hi32 = const.tile([P, NCH, D], f32)
        nc.vector.tensor_copy(rhs_hi[:, :, 0:D], e_f32[:])      # round to bf16
        nc.vector.tensor_copy(e_hi32[:], rhs_hi[:, :, 0:D])     # widen back
        nc.vector.tensor_sub(rhs_lo[:, :, 0:D], e_f32[:], e_hi32[:])
        nc.vector.memset(rhs_hi[:, :, D:NE], 1.0)
        nc.vector.memset(rhs_lo[:, :, D:NE], 0.0)

        # row = (g*group + f)*128 + p
        xg = x.rearrange("(g f p) v -> g p f v", p=P, f=group)
        yg = y.rearrange("(g f p) d -> g p f d", p=P, f=group)

        for g in range(ng):
            xb = xb_pool.tile([P, group, V], bf16)
            nc.gpsimd.dma_start(xb[:], xg[g])  # fp32 -> bf16 cast during DMA
            out_sb = out_pool.tile([P, group, D], f32)
            for f in range(group):
                pt = psum_t.tile([P, NCH, P], bf16)
                for c in range(NCH):
                    nc.tensor.transpose(pt[:, c, :], xb[:, f, c * P:(c + 1) * P], ident)
                xt = xt_pool.tile([P, NCH, P], bf16)
                # PSUM -> SBUF copyback on DVE (ACT trips walrus'
                # per-instruction sync-wait limit in this dependency pattern)
                nc.vector.tensor_copy(xt[:, 0:4, :], pt[:, 0:4, :])
                nc.vector.tensor_copy(xt[:, 4:NCH, :], pt[:, 4:NCH, :])
                po = psum_o.tile([P, NE], f32)
                for c in range(NCH):
                    nc.tensor.matmul(po[:], xt[:, c, :], rhs_hi[:, c, :],
                                     start=(c == 0), stop=False)
                    nc.tensor.matmul(po[:], xt[:, c, :], rhs_lo[:, c, :],
                                     start=False, stop=(c == NCH - 1))
                r = small.tile([P, 1], f32)
                nc.vector.tensor_scalar_max(r[:], po[:, D:NE], 1.0)
                nc.vector.reciprocal(r[:], r[:])
                nc.vector.tensor_scalar_mul(out_sb[:, f, :], po[:, 0:D], r[:])
            nc.sync.dma_start(yg[g], out_sb[:])

    return nc


_cached_nc = None


def kernel(**inputs):
    global _cached_nc
    from concourse.bass_utils import run_bass_kernel_spmd

    x = np.asarray(inputs["batch_vectors"], dtype=np.float32).reshape(B, S, V)
    e = np.ascontiguousarray(np.asarray(inputs["embedding_matrix"], dtype=np.float32))

    if _cached_nc is None:
        _cached_nc = build_kernel()

    in_maps = []
    for i in range(NCORES):
        shard = np.ascontiguousarray(
            x[i * PER_CORE_B:(i + 1) * PER_CORE_B].reshape(ROWS, V)
        )
        in_maps.append({"x": shard, "emb": e})

    res = run_bass_kernel_spmd(_cached_nc, in_maps, core_ids=list(range(NCORES)))
    out = np.concatenate(
        [res.results[i]["y"].reshape(PER_CORE_B, S, D) for i in range(NCORES)],
        axis=0,
    )
    return out.astype(np.float32)



# revision 9
# speedup vs baseline: 1.1999x; 1.1999x over previous
"""Trainium2 Bass kernel for nn_KC_Avg_Embedding (multi-hot averaged embedding).

Computes, for multi-hot indicator vectors x[b,s,:] over a vocabulary of 1024:
    out[b,s,:] = (x[b,s,:] @ E) / max(sum(x[b,s,:]), 1)

Strategy (data-parallel over 8 NeuronCores, batch-sharded):
  - Each core gets rows = (B/8)*S = 3200 rows of x plus the full embedding
    matrix E [1024, 128] fp32.
  - The shard is laid out vocab-major on the host (xT [1024, 3200] fp32) so the
    device DMA lands it directly with the vocab dim on partitions — no on-chip
    transposes and no PSUM->SBUF copybacks.
  - x is DMA'd in with an fp32->bf16 cast (SWDGE); x is 0/1 so bf16 is exact.
  - E is cast to bf16 (rel err ~2e-3, well inside the 2e-2 gate) and extended
    with a ones column; per 128-row tile, 8 accumulating matmuls (one per
    128-vocab chunk, stationary xT chunk, moving [E | 1]) produce
    [128 rows, 129] in PSUM = [x@E | row_count] with fp32 accumulation.
  - Epilogue: DVE computes 1/max(count,1); ACT scales the embedding columns.
"""

import sys
from contextlib import ExitStack

import numpy as np

for _p in ("/opt/trn_rl_repo",):
    if _p not in sys.path:
        sys.path.insert(0, _p)

import concourse.bass as bass
import concourse.mybir as mybir
import concourse.tile as tile

from concourse.vector_clock import ScopedClock


class _SplitDrainTC(tile.TileContext):
    """TileContext that splits multi-semaphore waits across single-wait
    carrier nops — this walrus build enforces a one-sync-wait-per-instruction
    codegen limit (matmul and activation structs reject 2+ waits)."""

    def _commit_instruction(self, inst, lazy_reg_writes: bool = True):
        si = getattr(inst, "sync_info", None)
        if (
            si is not None
            and si.on_wait
            and len(si.on_wait) > 1
            and inst.engine != mybir.EngineType.Unassigned
        ):
            waits = list(si.on_wait)
            del si.on_wait[1:]
            for w in waits[1:]:
                nop = mybir.InstNoOp(
                    name=self.nc.get_next_instruction_name(),
                    engine=inst.engine,
                    sync_info=mybir.SyncInfo(on_wait=[w], on_update=[]),
                    bass_nofuse=True,
                )
                super()._commit_instruction(nop, lazy_reg_writes)
        super()._commit_instruction(inst, lazy_reg_writes)

    def _drain_and_barrier(self, tick_clock, wait_clock):
        drain_inst = self.nc.sync.drain()
        wait_clock.add_sem_waits(
            drain_inst.ins, ScopedClock({None: tick_clock.global_clock})
        )
        si = drain_inst.ins.sync_info
        if si is not None and si.on_wait is not None and len(si.on_wait) > 1:
            waits = list(si.on_wait)
            del si.on_wait[1:]
            for w in waits[1:]:
                nop = self.nc.sync.nop(nofuse=True, hint="drain_wait_split")
                nsi = nop.ins.sync_info
                if nsi is None:
                    nop.ins.sync_info = mybir.SyncInfo(on_update=[], on_wait=[w])
                else:
                    nsi.on_wait.append(w)
        self.nc.all_engine_barrier()
        assert self.sems is not None
        popped = self.nc._tile_sem_poison_stack.pop()
        assert popped is self._sem_poison
        self.nc.clear_and_free_semaphores(list(self.sems.allocated().values()))
        self.nc.all_engine_barrier()


B, S, V, D = 128, 200, 1024, 128
NCORES = 8
P = 128
PER_CORE_B = B // NCORES          # 16
ROWS = PER_CORE_B * S             # 3200 rows per core
NT = ROWS // P                    # 25 row tiles per core
NCH = V // P                      # 8 vocab chunks
NE = D + 1                        # 128 emb cols + 1 count col
# row tiles per DMA group: small first group (fast ramp), small last (fast tail)
GROUPS = (3, 5, 5, 5, 5, 2)
assert sum(GROUPS) == NT


def build_kernel():
    nc = bass.Bass()
    # x arrives vocab-major: xT[v, r] = x[r, v]
    x = nc.declare_dram_parameter("x", [V, ROWS], mybir.dt.float32, isOutput=False)
    # emb arrives with a ones column appended on the host: [E | 1] so each
    # matmul also accumulates the row count, and rhs has a single DMA writer
    # (walrus caps sync-waits per instruction; a separate memset writer pushed
    # the first matmul over the cap).
    emb = nc.declare_dram_parameter("emb", [V, NE], mybir.dt.float32, isOutput=False)
    y = nc.declare_dram_parameter("y", [ROWS, D], mybir.dt.float32, isOutput=True)

    bf16 = mybir.dt.bfloat16
    f32 = mybir.dt.float32

    with _SplitDrainTC(nc) as tc, ExitStack() as ctx:
        const = ctx.enter_context(tc.tile_pool(name="const", bufs=1))
        xb_pool = ctx.enter_context(tc.tile_pool(name="xb", bufs=len(GROUPS)))
        out_pool = ctx.enter_context(tc.tile_pool(name="out", bufs=len(GROUPS)))
        small = ctx.enter_context(tc.tile_pool(name="small", bufs=NT))
        psum_o = ctx.enter_context(tc.tile_pool(name="psum_o", bufs=8, space="PSUM"))

        # [E | 1] -> [vocab-chunk partition, chunk, col] bf16; the trailing
        # ones column makes each matmul also accumulate the row count.
        rhs = const.tile([P, NCH, NE], bf16)
        nc.gpsimd.dma_start(rhs[:], emb.rearrange("(c p) d -> p c d", p=P))

        xv = x.rearrange("(c p) r -> p c r", p=P)     # [128, 8, 3200]
        yv = y.rearrange("(t p) d -> p t d", p=P)     # [128, 25, 128]

        t0 = 0
        for nt in GROUPS:
            r0 = t0 * P
            xb = xb_pool.tile([P, NCH, nt * P], bf16, name=f"xb{t0}", tag="xb")
            nc.gpsimd.dma_start(xb[:], xv[:, :, r0:r0 + nt * P])  # fp32->bf16 cast
            out_sb = out_pool.tile([P, nt, D], f32, name=f"out{t0}", tag="out")
            for f in range(nt):
                po = psum_o.tile([P, NE], f32, name=f"po{t0}_{f}", tag="po")
                for c in range(NCH):
                    nc.tensor.matmul(po[:], xb[:, c, f * P:(f + 1) * P],
                                     rhs[:, c, :],
                                     start=(c == 0), stop=(c == NCH - 1))
                r = small.tile([P, 1], f32, name=f"r{t0}_{f}", tag="r")
                nc.vector.tensor_scalar_max(r[:], po[:, D:NE], 1.0)
                nc.vector.reciprocal(r[:], r[:])
                # all-DVE epilogue: the mul's PE dep is implied by the max's
                # wait, so no instruction exceeds walrus' sync-wait cap (ACT
                # has a smaller cap and can't take this op).
                nc.vector.tensor_scalar_mul(out_sb[:, f, :], po[:, 0:D], r[:])
            nc.sync.dma_start(yv[:, t0:t0 + nt, :], out_sb[:])
            t0 += nt

    return nc


_cached_nc = None


def _get_nc():
    global _cached_nc
    if _cached_nc is None:
        _cached_nc = build_kernel()
    return _cached_nc


def make_in_maps(inputs):
    """Host-side shard prep: batch-shard x, lay each shard out vocab-major."""
    x = np.asarray(inputs["batch_vectors"], dtype=np.float32).reshape(NCORES, ROWS, V)
    e = np.asarray(inputs["embedding_matrix"], dtype=np.float32)
    e_aug = np.ascontiguousarray(
        np.concatenate([e, np.ones((V, 1), dtype=np.float32)], axis=1)
    )
    xt = np.ascontiguousarray(x.transpose(0, 2, 1))  # [8, 1024, 3200]
    return [{"x": xt[i], "emb": e_aug} for i in range(NCORES)]


def kernel(**inputs):
    from concourse.bass_utils import run_bass_kernel_spmd

    in_maps = make_in_maps(inputs)
    res = run_bass_kernel_spmd(_get_nc(), in_maps, core_ids=list(range(NCORES)))
    out = np.concatenate(
        [res.results[i]["y"].reshape(PER_CORE_B, S, D) for i in range(NCORES)],
        axis=0,
    )
    return out.astype(np.float32)


# revision 14
# speedup vs baseline: 1.6919x; 1.4101x over previous
"""Trainium2 Bass kernel for nn_KC_Avg_Embedding (multi-hot averaged embedding).

Computes, for multi-hot indicator vectors x[b,s,:] over a vocabulary of 1024:
    out[b,s,:] = (x[b,s,:] @ E) / max(sum(x[b,s,:]), 1)

Strategy (data-parallel over 8 NeuronCores, batch-sharded):
  - Each core gets rows = (B/8)*S = 3200 rows of x plus the full embedding
    matrix E [1024, 128].
  - The shard is staged vocab-major bf16 on the host (x is 0/1 so bf16 is
    exact) so the device DMA lands it with the vocab dim on partitions — no
    on-chip transposes, no casts, no PSUM->SBUF copybacks, and every DMA can
    use the fast HWDGE path.
  - E is host-rounded to bf16 (rel err ~2e-3, inside the 2e-2 gate) and
    host-packed to [128 part, 8 chunk, 129] with a ones column appended so
    each matmul also accumulates the row count.
  - E and the first x groups are issued as raw DMAs *before* the TileContext
    so the HBM stream runs under the fixed ~6us engine-boot/preamble cost;
    their consumers get semaphore waits attached at commit time (attaching to
    the instruction is what survives the tile scheduler's reordering).
  - x loads ride the Sync HWDGE ring, y stores the Scalar HWDGE ring —
    separate FIFOs, so a compute-blocked store can't head-of-line block the
    x stream.
  - Per 128-row tile: 8 accumulating matmuls (stationary xT chunk, moving
    [E | 1]) -> PSUM [128 rows, 129] = [x@E | count]; DVE epilogue computes
    1/max(count,1) and scales.
"""

import sys
from contextlib import ExitStack

import numpy as np

for _p in ("/opt/trn_rl_repo",):
    if _p not in sys.path:
        sys.path.insert(0, _p)

import concourse.bass as bass
import concourse.mybir as mybir
import concourse.tile as tile

from concourse.vector_clock import ScopedClock


class _SplitDrainTC(tile.TileContext):
    """TileContext tweaks for this walrus build:

    - attaches prefetch-DMA semaphore waits to the first committed instruction
      that references each prefetched tensor (standalone EventSemaphore waits
      get reordered past their consumers by the tile scheduler);
    - splits multi-semaphore waits across single-wait carrier nops (this
      walrus enforces a one-sync-wait-per-instruction codegen limit)."""

    def _commit_instruction(self, inst, lazy_reg_writes: bool = True):
        pw = getattr(self.nc, "_prefetch_waits", None)
        if pw and getattr(inst, "ins", None):
            for arg in list(inst.ins):
                t = getattr(arg, "tensor", None)
                if t is None:
                    bap = getattr(arg, "bass_ap", None)
                    t = getattr(bap, "tensor", None) if bap is not None else None
                nm = getattr(t, "name", None)
                if nm in pw:
                    sem, val = pw.pop(nm)
                    bass.BassInstruction(inst).wait_op(sem, val, "sem-ge")
        si = getattr(inst, "sync_info", None)
        if (
            si is not None
            and si.on_wait
            and len(si.on_wait) > 1
            and inst.engine != mybir.EngineType.Unassigned
        ):
            waits = list(si.on_wait)
            del si.on_wait[1:]
            for w in waits[1:]:
                nop = mybir.InstNoOp(
                    name=self.nc.get_next_instruction_name(),
                    engine=inst.engine,
                    sync_info=mybir.SyncInfo(on_wait=[w], on_update=[]),
                    bass_nofuse=True,
                )
                super()._commit_instruction(nop, lazy_reg_writes)
        super()._commit_instruction(inst, lazy_reg_writes)

    def _drain_and_barrier(self, tick_clock, wait_clock):
        drain_inst = self.nc.sync.drain()
        wait_clock.add_sem_waits(
            drain_inst.ins, ScopedClock({None: tick_clock.global_clock})
        )
        si = drain_inst.ins.sync_info
        if si is not None and si.on_wait is not None and len(si.on_wait) > 1:
            waits = list(si.on_wait)
            del si.on_wait[1:]
            for w in waits[1:]:
                nop = self.nc.sync.nop(nofuse=True, hint="drain_wait_split")
                nsi = nop.ins.sync_info
                if nsi is None:
                    nop.ins.sync_info = mybir.SyncInfo(on_update=[], on_wait=[w])
                else:
                    nsi.on_wait.append(w)
        self.nc.all_engine_barrier()
        assert self.sems is not None
        popped = self.nc._tile_sem_poison_stack.pop()
        assert popped is self._sem_poison
        self.nc.clear_and_free_semaphores(list(self.sems.allocated().values()))
        self.nc.all_engine_barrier()


B, S, V, D = 128, 200, 1024, 128
NCORES = 8
P = 128
PER_CORE_B = B // NCORES          # 16
ROWS = PER_CORE_B * S             # 3200 rows per core
NT = ROWS // P                    # 25 row tiles per core
NCH = V // P                      # 8 vocab chunks
NE = D + 1                        # 128 emb cols + 1 count col
# row tiles per DMA group: ramp up for fast first compute, ramp down so the
# last x bytes arrive when little work remains
GROUPS = (1, 2, 4, 5, 5, 4, 3, 1)
N_PREFETCH = 3                    # x groups issued as raw DMAs before tc entry
assert sum(GROUPS) == NT


def build_kernel():
    nc = bass.Bass()
    # x arrives vocab-major bf16: xT[v, r] = x[r, v]
    x = nc.declare_dram_parameter("x", [V, ROWS], mybir.dt.bfloat16, isOutput=False)
    # emb arrives host-packed bf16: [128 part, 8 chunk, 129] with ones column
    emb = nc.declare_dram_parameter(
        "emb", [P, NCH, NE], mybir.dt.bfloat16, isOutput=False
    )
    y = nc.declare_dram_parameter("y", [ROWS, D], mybir.dt.float32, isOutput=True)

    bf16 = mybir.dt.bfloat16
    f32 = mybir.dt.float32

    xv = x.rearrange("(c p) r -> p c r", p=P)     # [128, 8, 3200]
    yv = y.rearrange("(t p) d -> p t d", p=P)     # [128, 25, 128]

    bounds = []
    t0 = 0
    for nt in GROUPS:
        bounds.append((t0, nt))
        t0 += nt

    with ExitStack() as ctx:
        # Raw SBUF tensors + prefetch DMAs, emitted before the TileContext so
        # the HBM stream runs under the entry preamble. HWDGE (sync ring).
        rhs_t = ctx.enter_context(nc.sbuf_tensor([P, NCH, NE], bf16))
        rhs = rhs_t[:, :, :]
        pre_x = []
        for g in range(N_PREFETCH):
            tg, nt = bounds[g]
            h = ctx.enter_context(nc.sbuf_tensor([P, NCH, nt * P], bf16))
            pre_x.append(h[:, :, :])

        sems = [nc.alloc_semaphore(f"pref{i}") for i in range(1 + N_PREFETCH)]
        nc._prefetch_waits = {}
        for s in sems:
            nc.sync.sem_clear(s)
        nc.sync.dma_start(rhs, emb[:, :, :]).then_inc(sems[0], 16)
        nc._prefetch_waits[rhs_t.name] = (sems[0], 16)
        for g in range(N_PREFETCH):
            tg, nt = bounds[g]
            nc.sync.dma_start(
                pre_x[g], xv[:, :, tg * P:(tg + nt) * P]
            ).then_inc(sems[1 + g], 16)
            nc._prefetch_waits[pre_x[g].tensor.name] = (sems[1 + g], 16)

        with _SplitDrainTC(nc) as tc, ExitStack() as pools:
            xb_pool = pools.enter_context(
                tc.tile_pool(name="xb", bufs=len(GROUPS) - N_PREFETCH)
            )
            out_pool = pools.enter_context(tc.tile_pool(name="out", bufs=len(GROUPS)))
            small = pools.enter_context(tc.tile_pool(name="small", bufs=NT))
            psum_o = pools.enter_context(
                tc.tile_pool(name="psum_o", bufs=8, space="PSUM")
            )

            for g, (tg, nt) in enumerate(bounds):
                if g < N_PREFETCH:
                    xb = pre_x[g]
                else:
                    xb = xb_pool.tile([P, NCH, nt * P], bf16, name=f"xb{tg}", tag="xb")
                    nc.sync.dma_start(xb[:], xv[:, :, tg * P:(tg + nt) * P])
                out_sb = out_pool.tile([P, nt, D], f32, name=f"out{tg}", tag="out")
                for f in range(nt):
                    po = psum_o.tile([P, NE], f32, name=f"po{tg}_{f}", tag="po")
                    for c in range(NCH):
                        nc.tensor.matmul(po[:], xb[:, c, f * P:(f + 1) * P],
                                         rhs[:, c, :],
                                         start=(c == 0), stop=(c == NCH - 1))
                    r = small.tile([P, 1], f32, name=f"r{tg}_{f}", tag="r")
                    nc.vector.tensor_scalar_max(r[:], po[:, D:NE], 1.0)
                    nc.vector.reciprocal(r[:], r[:])
                    # all-DVE epilogue: the mul's PE dep is implied by the
                    # max's wait (ACT has a 1-wait cap and can't take this op)
                    nc.vector.tensor_scalar_mul(out_sb[:, f, :], po[:, 0:D], r[:])
                # y stores ride the Scalar HWDGE ring so they can't
                # head-of-line block the x loads on the Sync ring
                nc.scalar.dma_start(yv[:, tg:tg + nt, :], out_sb[:])

        nc._prefetch_waits = {}

    return nc


_cached_nc = None


def _get_nc():
    global _cached_nc
    if _cached_nc is None:
        _cached_nc = build_kernel()
    return _cached_nc


def _blocked_T(src):
    """[n, R, V] -> [n, V, R] contiguous, cache-blocked (3x faster than
    numpy's strided transpose copy on 100MB inputs)."""
    n, R, Vd = src.shape
    out = np.empty((n, Vd, R), src.dtype)
    Bk = 128
    for k in range(n):
        s, o = src[k], out[k]
        for i in range(0, R, Bk):
            for j in range(0, Vd, Bk):
                o[j:j + Bk, i:i + Bk] = s[i:i + Bk, j:j + Bk].T
    return out


def make_in_maps(inputs):
    """Host-side shard prep: batch-shard x vocab-major bf16 (x is 0/1 ->
    exact); pack [E | 1] bf16 partition-major."""
    import ml_dtypes

    bf16 = ml_dtypes.bfloat16
    x = np.asarray(inputs["batch_vectors"], dtype=np.float32).reshape(NCORES, ROWS, V)
    e = np.asarray(inputs["embedding_matrix"], dtype=np.float32)
    e_aug = np.concatenate([e, np.ones((V, 1), dtype=np.float32)], axis=1)
    e_dev = np.ascontiguousarray(
        e_aug.reshape(NCH, P, NE).transpose(1, 0, 2).astype(bf16)
    )
    xt = _blocked_T(x.astype(bf16))  # [8, 1024, 3200] bf16
    return [{"x": xt[i], "emb": e_dev} for i in range(NCORES)]


def kernel(**inputs):
    from concourse.bass_utils import run_bass_kernel_spmd

    in_maps = make_in_maps(inputs)
    res = run_bass_kernel_spmd(_get_nc(), in_maps, core_ids=list(range(NCORES)))
    out = np.concatenate(
        [res.results[i]["y"].reshape(PER_CORE_B, S, D) for i in range(NCORES)],
        axis=0,
    )
    return out.astype(np.float32)


# revision 16
# speedup vs baseline: 1.7346x; 1.0252x over previous
"""Trainium2 Bass kernel for nn_KC_Avg_Embedding (multi-hot averaged embedding).

Computes, for multi-hot indicator vectors x[b,s,:] over a vocabulary of 1024:
    out[b,s,:] = (x[b,s,:] @ E) / max(sum(x[b,s,:]), 1)

Strategy (data-parallel over 8 NeuronCores, batch-sharded):
  - Each core gets rows = (B/8)*S = 3200 rows of x plus the full embedding
    matrix E [1024, 128].
  - The shard is staged vocab-major bf16 on the host (x is 0/1 so bf16 is
    exact) so the device DMA lands it with the vocab dim on partitions — no
    on-chip transposes, no casts, no PSUM->SBUF copybacks, and every DMA can
    use the fast HWDGE path.
  - E is host-rounded to bf16 (rel err ~2e-3, inside the 2e-2 gate) and
    host-packed to [128 part, 8 chunk, 129] with a ones column appended so
    each matmul also accumulates the row count.
  - E and the first x groups are issued as raw DMAs *before* the TileContext
    so the HBM stream runs under the fixed ~6us engine-boot/preamble cost;
    their consumers get semaphore waits attached at commit time (attaching to
    the instruction is what survives the tile scheduler's reordering).
  - x loads ride the Sync HWDGE ring, y stores the Scalar HWDGE ring —
    separate FIFOs, so a compute-blocked store can't head-of-line block the
    x stream.
  - Per 128-row tile: 8 accumulating matmuls (stationary xT chunk, moving
    [E | 1]) -> PSUM [128 rows, 129] = [x@E | count]; DVE epilogue computes
    1/max(count,1) and scales.
"""

import sys
from contextlib import ExitStack

import numpy as np

for _p in ("/opt/trn_rl_repo",):
    if _p not in sys.path:
        sys.path.insert(0, _p)

import concourse.bass as bass
import concourse.mybir as mybir
import concourse.tile as tile

from concourse.vector_clock import ScopedClock


class _SplitDrainTC(tile.TileContext):
    """TileContext tweaks for this walrus build:

    - attaches prefetch-DMA semaphore waits to the first committed instruction
      that references each prefetched tensor (standalone EventSemaphore waits
      get reordered past their consumers by the tile scheduler);
    - splits multi-semaphore waits across single-wait carrier nops (this
      walrus enforces a one-sync-wait-per-instruction codegen limit)."""

    def _commit_instruction(self, inst, lazy_reg_writes: bool = True):
        pw = getattr(self.nc, "_prefetch_waits", None)
        if pw and getattr(inst, "ins", None):
            for arg in list(inst.ins):
                t = getattr(arg, "tensor", None)
                if t is None:
                    bap = getattr(arg, "bass_ap", None)
                    t = getattr(bap, "tensor", None) if bap is not None else None
                nm = getattr(t, "name", None)
                if nm in pw:
                    sem, val = pw.pop(nm)
                    bass.BassInstruction(inst).wait_op(sem, val, "sem-ge")
        si = getattr(inst, "sync_info", None)
        if (
            si is not None
            and si.on_wait
            and len(si.on_wait) > 1
            and inst.engine != mybir.EngineType.Unassigned
        ):
            waits = list(si.on_wait)
            del si.on_wait[1:]
            for w in waits[1:]:
                nop = mybir.InstNoOp(
                    name=self.nc.get_next_instruction_name(),
                    engine=inst.engine,
                    sync_info=mybir.SyncInfo(on_wait=[w], on_update=[]),
                    bass_nofuse=True,
                )
                super()._commit_instruction(nop, lazy_reg_writes)
        super()._commit_instruction(inst, lazy_reg_writes)

    def _drain_and_barrier(self, tick_clock, wait_clock):
        drain_inst = self.nc.sync.drain()
        wait_clock.add_sem_waits(
            drain_inst.ins, ScopedClock({None: tick_clock.global_clock})
        )
        si = drain_inst.ins.sync_info
        if si is not None and si.on_wait is not None and len(si.on_wait) > 1:
            waits = list(si.on_wait)
            del si.on_wait[1:]
            for w in waits[1:]:
                nop = self.nc.sync.nop(nofuse=True, hint="drain_wait_split")
                nsi = nop.ins.sync_info
                if nsi is None:
                    nop.ins.sync_info = mybir.SyncInfo(on_update=[], on_wait=[w])
                else:
                    nsi.on_wait.append(w)
        self.nc.all_engine_barrier()
        assert self.sems is not None
        popped = self.nc._tile_sem_poison_stack.pop()
        assert popped is self._sem_poison
        self.nc.clear_and_free_semaphores(list(self.sems.allocated().values()))
        self.nc.all_engine_barrier()


B, S, V, D = 128, 200, 1024, 128
NCORES = 8
P = 128
PER_CORE_B = B // NCORES          # 16
ROWS = PER_CORE_B * S             # 3200 rows per core
NT = ROWS // P                    # 25 row tiles per core
NCH = V // P                      # 8 vocab chunks
NE = D + 1                        # 128 emb cols + 1 count col
# row tiles per DMA group: ramp up for fast first compute, ramp down so the
# last x bytes arrive when little work remains
GROUPS = (1, 2, 4, 5, 5, 4, 3, 1)
N_PREFETCH = 3                    # x groups issued as raw DMAs before tc entry
assert sum(GROUPS) == NT


def build_kernel():
    nc = bass.Bass()
    # x arrives vocab-major bf16: xT[v, r] = x[r, v]
    x = nc.declare_dram_parameter("x", [V, ROWS], mybir.dt.bfloat16, isOutput=False)
    # emb arrives host-packed bf16: [128 part, 8 chunk, 129] with ones column
    emb = nc.declare_dram_parameter(
        "emb", [P, NCH, NE], mybir.dt.bfloat16, isOutput=False
    )
    # y is stored bf16 (adds ~2e-3 rel err on top of bf16-E's ~1.7e-3; gate is
    # 2e-2) and widened to fp32 on the host — halves the store traffic.
    y = nc.declare_dram_parameter("y", [ROWS, D], mybir.dt.bfloat16, isOutput=True)

    bf16 = mybir.dt.bfloat16
    f32 = mybir.dt.float32

    xv = x.rearrange("(c p) r -> p c r", p=P)     # [128, 8, 3200]
    yv = y.rearrange("(t p) d -> p t d", p=P)     # [128, 25, 128]

    bounds = []
    t0 = 0
    for nt in GROUPS:
        bounds.append((t0, nt))
        t0 += nt

    with ExitStack() as ctx:
        # Raw SBUF tensors + prefetch DMAs, emitted before the TileContext so
        # the HBM stream runs under the entry preamble. HWDGE (sync ring).
        rhs_t = ctx.enter_context(nc.sbuf_tensor([P, NCH, NE], bf16))
        rhs = rhs_t[:, :, :]
        pre_x = []
        for g in range(N_PREFETCH):
            tg, nt = bounds[g]
            h = ctx.enter_context(nc.sbuf_tensor([P, NCH, nt * P], bf16))
            pre_x.append(h[:, :, :])

        sems = [nc.alloc_semaphore(f"pref{i}") for i in range(1 + N_PREFETCH)]
        nc._prefetch_waits = {}
        for s in sems:
            nc.sync.sem_clear(s)
        nc.sync.dma_start(rhs, emb[:, :, :]).then_inc(sems[0], 16)
        nc._prefetch_waits[rhs_t.name] = (sems[0], 16)
        for g in range(N_PREFETCH):
            tg, nt = bounds[g]
            nc.sync.dma_start(
                pre_x[g], xv[:, :, tg * P:(tg + nt) * P]
            ).then_inc(sems[1 + g], 16)
            nc._prefetch_waits[pre_x[g].tensor.name] = (sems[1 + g], 16)

        with _SplitDrainTC(nc) as tc, ExitStack() as pools:
            xb_pool = pools.enter_context(
                tc.tile_pool(name="xb", bufs=len(GROUPS) - N_PREFETCH)
            )
            out_pool = pools.enter_context(tc.tile_pool(name="out", bufs=len(GROUPS)))
            small = pools.enter_context(tc.tile_pool(name="small", bufs=NT))
            psum_o = pools.enter_context(
                tc.tile_pool(name="psum_o", bufs=8, space="PSUM")
            )

            for g, (tg, nt) in enumerate(bounds):
                if g < N_PREFETCH:
                    xb = pre_x[g]
                else:
                    xb = xb_pool.tile([P, NCH, nt * P], bf16, name=f"xb{tg}", tag="xb")
                    nc.sync.dma_start(xb[:], xv[:, :, tg * P:(tg + nt) * P])
                out_sb = out_pool.tile([P, nt, D], bf16, name=f"out{tg}", tag="out")
                for f in range(nt):
                    po = psum_o.tile([P, NE], f32, name=f"po{tg}_{f}", tag="po")
                    for c in range(NCH):
                        nc.tensor.matmul(po[:], xb[:, c, f * P:(f + 1) * P],
                                         rhs[:, c, :],
                                         start=(c == 0), stop=(c == NCH - 1))
                    r = small.tile([P, 1], f32, name=f"r{tg}_{f}", tag="r")
                    nc.vector.tensor_scalar_max(r[:], po[:, D:NE], 1.0)
                    nc.vector.reciprocal(r[:], r[:])
                    # count/reciprocal on DVE; the [128,128] scale-mul on the
                    # otherwise-idle ACT engine (DVE saturated to ~90% when it
                    # carried the mul too; the commit-time wait splitter covers
                    # ACT's 1-wait codegen cap)
                    nc.scalar.mul(out_sb[:, f, :], po[:, 0:D], r[:])
                # y stores ride the Scalar HWDGE ring so they can't
                # head-of-line block the x loads on the Sync ring
                nc.scalar.dma_start(yv[:, tg:tg + nt, :], out_sb[:])

        nc._prefetch_waits = {}

    return nc


_cached_nc = None


def _get_nc():
    global _cached_nc
    if _cached_nc is None:
        _cached_nc = build_kernel()
    return _cached_nc


def _blocked_T(src):
    """[n, R, V] -> [n, V, R] contiguous, cache-blocked (3x faster than
    numpy's strided transpose copy on 100MB inputs)."""
    n, R, Vd = src.shape
    out = np.empty((n, Vd, R), src.dtype)
    Bk = 128
    for k in range(n):
        s, o = src[k], out[k]
        for i in range(0, R, Bk):
            for j in range(0, Vd, Bk):
                o[j:j + Bk, i:i + Bk] = s[i:i + Bk, j:j + Bk].T
    return out


def make_in_maps(inputs):
    """Host-side shard prep: batch-shard x vocab-major bf16 (x is 0/1 ->
    exact); pack [E | 1] bf16 partition-major."""
    import ml_dtypes

    bf16 = ml_dtypes.bfloat16
    x = np.asarray(inputs["batch_vectors"], dtype=np.float32).reshape(NCORES, ROWS, V)
    e = np.asarray(inputs["embedding_matrix"], dtype=np.float32)
    e_aug = np.concatenate([e, np.ones((V, 1), dtype=np.float32)], axis=1)
    e_dev = np.ascontiguousarray(
        e_aug.reshape(NCH, P, NE).transpose(1, 0, 2).astype(bf16)
    )
    xt = _blocked_T(x.astype(bf16))  # [8, 1024, 3200] bf16
    return [{"x": xt[i], "emb": e_dev} for i in range(NCORES)]


def kernel(**inputs):
    from concourse.bass_utils import run_bass_kernel_spmd

    in_maps = make_in_maps(inputs)
    res = run_bass_kernel_spmd(_get_nc(), in_maps, core_ids=list(range(NCORES)))
    out = np.concatenate(
        [res.results[i]["y"].reshape(PER_CORE_B, S, D) for i in range(NCORES)],
        axis=0,
    )
    return out.astype(np.float32)


# revision 18
# speedup vs baseline: 1.7731x; 1.0222x over previous
"""Trainium2 Bass kernel for nn_KC_Avg_Embedding (multi-hot averaged embedding).

Computes, for multi-hot indicator vectors x[b,s,:] over a vocabulary of 1024:
    out[b,s,:] = (x[b,s,:] @ E) / max(sum(x[b,s,:]), 1)

Strategy (data-parallel over 8 NeuronCores, batch-sharded):
  - Each core gets rows = (B/8)*S = 3200 rows of x plus the full embedding
    matrix E [1024, 128].
  - The shard is staged vocab-major bf16 on the host (x is 0/1 so bf16 is
    exact) so the device DMA lands it with the vocab dim on partitions — no
    on-chip transposes, no casts, no PSUM->SBUF copybacks, and every DMA can
    use the fast HWDGE path.
  - E is host-rounded to bf16 (rel err ~2e-3, inside the 2e-2 gate) and
    host-packed to [128 part, 8 chunk, 129] with a ones column appended so
    each matmul also accumulates the row count.
  - E and the first x groups are issued as raw DMAs *before* the TileContext
    so the HBM stream runs under the fixed ~6us engine-boot/preamble cost;
    their consumers get semaphore waits attached at commit time (attaching to
    the instruction is what survives the tile scheduler's reordering).
  - x loads ride the Sync HWDGE ring, y stores the Scalar HWDGE ring —
    separate FIFOs, so a compute-blocked store can't head-of-line block the
    x stream.
  - Per 128-row tile: 8 accumulating matmuls (stationary xT chunk, moving
    [E | 1]) -> PSUM [128 rows, 129] = [x@E | count]; DVE epilogue computes
    1/max(count,1) and scales.
"""

import sys
from contextlib import ExitStack

import numpy as np

for _p in ("/opt/trn_rl_repo",):
    if _p not in sys.path:
        sys.path.insert(0, _p)

import concourse.bass as bass
import concourse.mybir as mybir
import concourse.tile as tile

from concourse.vector_clock import ScopedClock


class _SplitDrainTC(tile.TileContext):
    """TileContext tweaks for this walrus build:

    - attaches prefetch-DMA semaphore waits to the first committed instruction
      that references each prefetched tensor (standalone EventSemaphore waits
      get reordered past their consumers by the tile scheduler);
    - splits multi-semaphore waits across single-wait carrier nops (this
      walrus enforces a one-sync-wait-per-instruction codegen limit)."""

    def _commit_instruction(self, inst, lazy_reg_writes: bool = True):
        pw = getattr(self.nc, "_prefetch_waits", None)
        if pw and getattr(inst, "ins", None):
            for arg in list(inst.ins):
                t = getattr(arg, "tensor", None)
                if t is None:
                    bap = getattr(arg, "bass_ap", None)
                    t = getattr(bap, "tensor", None) if bap is not None else None
                nm = getattr(t, "name", None)
                if nm in pw:
                    sem, val = pw.pop(nm)
                    bass.BassInstruction(inst).wait_op(sem, val, "sem-ge")
        si = getattr(inst, "sync_info", None)
        if (
            si is not None
            and si.on_wait
            and len(si.on_wait) > 1
            and inst.engine != mybir.EngineType.Unassigned
        ):
            waits = list(si.on_wait)
            del si.on_wait[1:]
            for w in waits[1:]:
                nop = mybir.InstNoOp(
                    name=self.nc.get_next_instruction_name(),
                    engine=inst.engine,
                    sync_info=mybir.SyncInfo(on_wait=[w], on_update=[]),
                    bass_nofuse=True,
                )
                super()._commit_instruction(nop, lazy_reg_writes)
        super()._commit_instruction(inst, lazy_reg_writes)

    def _drain_and_barrier(self, tick_clock, wait_clock):
        drain_inst = self.nc.sync.drain()
        wait_clock.add_sem_waits(
            drain_inst.ins, ScopedClock({None: tick_clock.global_clock})
        )
        si = drain_inst.ins.sync_info
        if si is not None and si.on_wait is not None and len(si.on_wait) > 1:
            waits = list(si.on_wait)
            del si.on_wait[1:]
            for w in waits[1:]:
                nop = self.nc.sync.nop(nofuse=True, hint="drain_wait_split")
                nsi = nop.ins.sync_info
                if nsi is None:
                    nop.ins.sync_info = mybir.SyncInfo(on_update=[], on_wait=[w])
                else:
                    nsi.on_wait.append(w)
        self.nc.all_engine_barrier()
        assert self.sems is not None
        popped = self.nc._tile_sem_poison_stack.pop()
        assert popped is self._sem_poison
        self.nc.clear_and_free_semaphores(list(self.sems.allocated().values()))
        self.nc.all_engine_barrier()


B, S, V, D = 128, 200, 1024, 128
NCORES = 8
P = 128
PER_CORE_B = B // NCORES          # 16
ROWS = PER_CORE_B * S             # 3200 rows per core
NT = ROWS // P                    # 25 row tiles per core
NCH = V // P                      # 8 vocab chunks
NE = D + 1                        # 128 emb cols + 1 count col
# row tiles per DMA group: ramp up for fast first compute, ramp down so the
# last x bytes arrive when little work remains
GROUPS = (1, 2, 4, 5, 5, 4, 3, 1)
N_PREFETCH = 3                    # x groups issued as raw DMAs before tc entry
assert sum(GROUPS) == NT


def build_kernel():
    nc = bass.Bass()
    # x arrives vocab-major bf16: xT[v, r] = x[r, v]
    x = nc.declare_dram_parameter("x", [V, ROWS], mybir.dt.bfloat16, isOutput=False)
    # emb arrives host-packed bf16: [128 part, 8 chunk, 129] with ones column
    emb = nc.declare_dram_parameter(
        "emb", [P, NCH, NE], mybir.dt.bfloat16, isOutput=False
    )
    # y is stored bf16 (adds ~2e-3 rel err on top of bf16-E's ~1.7e-3; gate is
    # 2e-2) and widened to fp32 on the host — halves the store traffic.
    y = nc.declare_dram_parameter("y", [ROWS, D], mybir.dt.bfloat16, isOutput=True)

    bf16 = mybir.dt.bfloat16
    f32 = mybir.dt.float32

    xv = x.rearrange("(c p) r -> p c r", p=P)     # [128, 8, 3200]
    yv = y.rearrange("(t p) d -> p t d", p=P)     # [128, 25, 128]

    bounds = []
    t0 = 0
    for nt in GROUPS:
        bounds.append((t0, nt))
        t0 += nt

    with ExitStack() as ctx:
        # Raw SBUF tensors + prefetch DMAs, emitted before the TileContext so
        # the HBM stream runs under the entry preamble. HWDGE (sync ring).
        rhs_t = ctx.enter_context(nc.sbuf_tensor([P, NCH, NE], bf16))
        rhs = rhs_t[:, :, :]
        pre_x = []
        for g in range(N_PREFETCH):
            tg, nt = bounds[g]
            h = ctx.enter_context(nc.sbuf_tensor([P, NCH, nt * P], bf16))
            pre_x.append(h[:, :, :])

        sems = [nc.alloc_semaphore(f"pref{i}") for i in range(1 + N_PREFETCH)]
        nc._prefetch_waits = {}
        for s in sems:
            nc.sync.sem_clear(s)
        nc.sync.dma_start(rhs, emb[:, :, :]).then_inc(sems[0], 16)
        nc._prefetch_waits[rhs_t.name] = (sems[0], 16)
        for g in range(N_PREFETCH):
            tg, nt = bounds[g]
            nc.sync.dma_start(
                pre_x[g], xv[:, :, tg * P:(tg + nt) * P]
            ).then_inc(sems[1 + g], 16)
            nc._prefetch_waits[pre_x[g].tensor.name] = (sems[1 + g], 16)

        with _SplitDrainTC(nc) as tc, ExitStack() as pools:
            xb_pool = pools.enter_context(
                tc.tile_pool(name="xb", bufs=len(GROUPS) - N_PREFETCH)
            )
            out_pool = pools.enter_context(tc.tile_pool(name="out", bufs=len(GROUPS)))
            small = pools.enter_context(tc.tile_pool(name="small", bufs=12))
            psum_o = pools.enter_context(
                tc.tile_pool(name="psum_o", bufs=8, space="PSUM")
            )

            for g, (tg, nt) in enumerate(bounds):
                if g < N_PREFETCH:
                    xb = pre_x[g]
                else:
                    xb = xb_pool.tile([P, NCH, nt * P], bf16, name=f"xb{tg}", tag="xb")
                    nc.sync.dma_start(xb[:], xv[:, :, tg * P:(tg + nt) * P])
                out_sb = out_pool.tile([P, nt, D], bf16, name=f"out{tg}", tag="out")
                # Batch up to 3 row tiles per PSUM bank (3x516B <= 2KB) so the
                # epilogue is one max+recip+broadcast-mul per batch instead of
                # per tile — a per-tile epilogue saturated DVE/ACT and
                # backpressured the matmul pipeline through PSUM-slot reuse.
                for b0 in range(0, nt, 3):
                    bsz = min(3, nt - b0)
                    po = psum_o.tile([P, 3, NE], f32, name=f"po{tg}_{b0}", tag="po")
                    for j in range(bsz):
                        f = b0 + j
                        for c in range(NCH):
                            nc.tensor.matmul(po[:, j, :],
                                             xb[:, c, f * P:(f + 1) * P],
                                             rhs[:, c, :],
                                             start=(c == 0), stop=(c == NCH - 1))
                    rb = small.tile([P, 3, 1], f32, name=f"r{tg}_{b0}", tag="r")
                    nc.vector.tensor_scalar_max(rb[:, 0:bsz, :], po[:, 0:bsz, D:NE], 1.0)
                    nc.vector.reciprocal(rb[:, 0:bsz, :], rb[:, 0:bsz, :])
                    nc.vector.tensor_mul(
                        out_sb[:, b0:b0 + bsz, :],
                        po[:, 0:bsz, 0:D],
                        rb[:, 0:bsz, :].to_broadcast([P, bsz, D]),
                    )
                # y stores ride the Scalar HWDGE ring so they can't
                # head-of-line block the x loads on the Sync ring
                nc.scalar.dma_start(yv[:, tg:tg + nt, :], out_sb[:])

        nc._prefetch_waits = {}

    return nc


_cached_nc = None


def _get_nc():
    global _cached_nc
    if _cached_nc is None:
        _cached_nc = build_kernel()
    return _cached_nc


def _blocked_T(src):
    """[n, R, V] -> [n, V, R] contiguous, cache-blocked (3x faster than
    numpy's strided transpose copy on 100MB inputs)."""
    n, R, Vd = src.shape
    out = np.empty((n, Vd, R), src.dtype)
    Bk = 128
    for k in range(n):
        s, o = src[k], out[k]
        for i in range(0, R, Bk):
            for j in range(0, Vd, Bk):
                o[j:j + Bk, i:i + Bk] = s[i:i + Bk, j:j + Bk].T
    return out


def make_in_maps(inputs):
    """Host-side shard prep: batch-shard x vocab-major bf16 (x is 0/1 ->
    exact); pack [E | 1] bf16 partition-major."""
    import ml_dtypes

    bf16 = ml_dtypes.bfloat16
    x = np.asarray(inputs["batch_vectors"], dtype=np.float32).reshape(NCORES, ROWS, V)
    e = np.asarray(inputs["embedding_matrix"], dtype=np.float32)
    e_aug = np.concatenate([e, np.ones((V, 1), dtype=np.float32)], axis=1)
    e_dev = np.ascontiguousarray(
        e_aug.reshape(NCH, P, NE).transpose(1, 0, 2).astype(bf16)
    )
    xt = _blocked_T(x.astype(bf16))  # [8, 1024, 3200] bf16
    return [{"x": xt[i], "emb": e_dev} for i in range(NCORES)]


def kernel(**inputs):
    from concourse.bass_utils import run_bass_kernel_spmd

    in_maps = make_in_maps(inputs)
    res = run_bass_kernel_spmd(_get_nc(), in_maps, core_ids=list(range(NCORES)))
    out = np.concatenate(
        [res.results[i]["y"].reshape(PER_CORE_B, S, D) for i in range(NCORES)],
        axis=0,
    )
    return out.astype(np.float32)


# revision 20
# speedup vs baseline: 2.0174x; 1.1378x over previous
"""Trainium2 Bass kernel for nn_KC_Avg_Embedding (multi-hot averaged embedding).

Computes, for multi-hot indicator vectors x[b,s,:] over a vocabulary of 1024:
    out[b,s,:] = (x[b,s,:] @ E) / max(sum(x[b,s,:]), 1)

Strategy (data-parallel over 8 NeuronCores, batch-sharded):
  - Each core gets rows = (B/8)*S = 3200 rows of x plus the full embedding
    matrix E [1024, 128].
  - The shard is staged vocab-major bf16 on the host (x is 0/1 so bf16 is
    exact) so the device DMA lands it with the vocab dim on partitions — no
    on-chip transposes, no casts, no PSUM->SBUF copybacks, and every DMA can
    use the fast HWDGE path.
  - E is host-rounded to bf16 (rel err ~2e-3, inside the 2e-2 gate) and
    host-packed to [128 part, 8 chunk, 129] with a ones column appended so
    each matmul also accumulates the row count.
  - E and the first x groups are issued as raw DMAs *before* the TileContext
    so the HBM stream runs under the fixed ~6us engine-boot/preamble cost;
    their consumers get semaphore waits attached at commit time (attaching to
    the instruction is what survives the tile scheduler's reordering).
  - x loads ride the Sync HWDGE ring, y stores the Scalar HWDGE ring —
    separate FIFOs, so a compute-blocked store can't head-of-line block the
    x stream.
  - Per 128-row tile: 8 accumulating matmuls (stationary xT chunk, moving
    [E | 1]) -> PSUM [128 rows, 129] = [x@E | count]; DVE epilogue computes
    1/max(count,1) and scales.
"""

import sys
from contextlib import ExitStack

import numpy as np

for _p in ("/opt/trn_rl_repo",):
    if _p not in sys.path:
        sys.path.insert(0, _p)

import concourse.bass as bass
import concourse.mybir as mybir
import concourse.tile as tile

from concourse.vector_clock import ScopedClock


class _SplitDrainTC(tile.TileContext):
    """TileContext tweaks for this walrus build:

    - attaches prefetch-DMA semaphore waits to the first committed instruction
      that references each prefetched tensor (standalone EventSemaphore waits
      get reordered past their consumers by the tile scheduler);
    - splits multi-semaphore waits across single-wait carrier nops (this
      walrus enforces a one-sync-wait-per-instruction codegen limit)."""

    def _commit_instruction(self, inst, lazy_reg_writes: bool = True):
        pw = getattr(self.nc, "_prefetch_waits", None)
        if pw and getattr(inst, "ins", None):
            for arg in list(inst.ins):
                t = getattr(arg, "tensor", None)
                if t is None:
                    bap = getattr(arg, "bass_ap", None)
                    t = getattr(bap, "tensor", None) if bap is not None else None
                nm = getattr(t, "name", None)
                if nm in pw:
                    sem, val = pw.pop(nm)
                    bass.BassInstruction(inst).wait_op(sem, val, "sem-ge")
        si = getattr(inst, "sync_info", None)
        if (
            si is not None
            and si.on_wait
            and len(si.on_wait) > 1
            and inst.engine != mybir.EngineType.Unassigned
        ):
            waits = list(si.on_wait)
            del si.on_wait[1:]
            for w in waits[1:]:
                nop = mybir.InstNoOp(
                    name=self.nc.get_next_instruction_name(),
                    engine=inst.engine,
                    sync_info=mybir.SyncInfo(on_wait=[w], on_update=[]),
                    bass_nofuse=True,
                )
                super()._commit_instruction(nop, lazy_reg_writes)
        super()._commit_instruction(inst, lazy_reg_writes)

    def _drain_and_barrier(self, tick_clock, wait_clock):
        drain_inst = self.nc.sync.drain()
        wait_clock.add_sem_waits(
            drain_inst.ins, ScopedClock({None: tick_clock.global_clock})
        )
        si = drain_inst.ins.sync_info
        if si is not None and si.on_wait is not None and len(si.on_wait) > 1:
            waits = list(si.on_wait)
            del si.on_wait[1:]
            for w in waits[1:]:
                nop = self.nc.sync.nop(nofuse=True, hint="drain_wait_split")
                nsi = nop.ins.sync_info
                if nsi is None:
                    nop.ins.sync_info = mybir.SyncInfo(on_update=[], on_wait=[w])
                else:
                    nsi.on_wait.append(w)
        self.nc.all_engine_barrier()
        assert self.sems is not None
        popped = self.nc._tile_sem_poison_stack.pop()
        assert popped is self._sem_poison
        self.nc.clear_and_free_semaphores(list(self.sems.allocated().values()))
        self.nc.all_engine_barrier()


B, S, V, D = 128, 200, 1024, 128
NCORES = 8
P = 128
PER_CORE_B = B // NCORES          # 16
ROWS = PER_CORE_B * S             # 3200 rows per core
NT = ROWS // P                    # 25 row tiles per core
NCH = V // P                      # 8 vocab chunks
NE = D + 1                        # 128 emb cols + 1 count col
# row tiles per DMA group: ramp up for fast first compute, ramp down so the
# last x bytes arrive when little work remains
GROUPS = (1, 2, 4, 5, 5, 4, 3, 1)
N_PREFETCH = 3                    # x groups issued as raw DMAs before tc entry
assert sum(GROUPS) == NT


def build_kernel():
    nc = bass.Bass()
    # x arrives vocab-major bf16: xT[v, r] = x[r, v]
    x = nc.declare_dram_parameter("x", [V, ROWS], mybir.dt.float8e4, isOutput=False)
    # emb arrives host-packed bf16: [128 part, 8 chunk, 129] with ones column
    emb = nc.declare_dram_parameter(
        "emb", [P, NCH, NE], mybir.dt.bfloat16, isOutput=False
    )
    # y is stored bf16 (adds ~2e-3 rel err on top of bf16-E's ~1.7e-3; gate is
    # 2e-2) and widened to fp32 on the host — halves the store traffic.
    y = nc.declare_dram_parameter("y", [ROWS, D], mybir.dt.bfloat16, isOutput=True)

    bf16 = mybir.dt.bfloat16
    fp8 = mybir.dt.float8e4
    f32 = mybir.dt.float32

    xv = x.rearrange("(c p) r -> p c r", p=P)     # [128, 8, 3200]
    yv = y.rearrange("(t p) d -> p t d", p=P)     # [128, 25, 128]

    bounds = []
    t0 = 0
    for nt in GROUPS:
        bounds.append((t0, nt))
        t0 += nt

    with ExitStack() as ctx:
        # Raw SBUF tensors + prefetch DMAs, emitted before the TileContext so
        # the HBM stream runs under the entry preamble. HWDGE (sync ring).
        rhs_t = ctx.enter_context(nc.sbuf_tensor([P, NCH, NE], bf16))
        rhs = rhs_t[:, :, :]
        pre_x = []
        for g in range(N_PREFETCH):
            tg, nt = bounds[g]
            h = ctx.enter_context(nc.sbuf_tensor([P, NCH, nt * P], fp8))
            pre_x.append(h[:, :, :])

        sems = [nc.alloc_semaphore(f"pref{i}") for i in range(1 + N_PREFETCH)]
        nc._prefetch_waits = {}
        for s in sems:
            nc.sync.sem_clear(s)
        nc.sync.dma_start(rhs, emb[:, :, :]).then_inc(sems[0], 16)
        nc._prefetch_waits[rhs_t.name] = (sems[0], 16)
        for g in range(N_PREFETCH):
            tg, nt = bounds[g]
            nc.sync.dma_start(
                pre_x[g], xv[:, :, tg * P:(tg + nt) * P]
            ).then_inc(sems[1 + g], 16)
            nc._prefetch_waits[pre_x[g].tensor.name] = (sems[1 + g], 16)

        with _SplitDrainTC(nc) as tc, ExitStack() as pools:
            xb_pool = pools.enter_context(
                tc.tile_pool(name="xb", bufs=len(GROUPS) - N_PREFETCH)
            )
            out_pool = pools.enter_context(tc.tile_pool(name="out", bufs=len(GROUPS)))
            small = pools.enter_context(tc.tile_pool(name="small", bufs=12))
            psum_o = pools.enter_context(
                tc.tile_pool(name="psum_o", bufs=8, space="PSUM")
            )

            for g, (tg, nt) in enumerate(bounds):
                if g < N_PREFETCH:
                    xb = pre_x[g]
                else:
                    xb = xb_pool.tile([P, NCH, nt * P], fp8, name=f"xb{tg}", tag="xb")
                    nc.sync.dma_start(xb[:], xv[:, :, tg * P:(tg + nt) * P])
                out_sb = out_pool.tile([P, nt, D], bf16, name=f"out{tg}", tag="out")
                # Batch up to 3 row tiles per PSUM bank (3x516B <= 2KB) so the
                # epilogue is one max+recip+broadcast-mul per batch instead of
                # per tile — a per-tile epilogue saturated DVE/ACT and
                # backpressured the matmul pipeline through PSUM-slot reuse.
                for b0 in range(0, nt, 3):
                    bsz = min(3, nt - b0)
                    po = psum_o.tile([P, 3, NE], f32, name=f"po{tg}_{b0}", tag="po")
                    for j in range(bsz):
                        f = b0 + j
                        for c in range(NCH):
                            nc.tensor.matmul(po[:, j, :],
                                             xb[:, c, f * P:(f + 1) * P],
                                             rhs[:, c, :],
                                             start=(c == 0), stop=(c == NCH - 1))
                    rb = small.tile([P, 3, 1], f32, name=f"r{tg}_{b0}", tag="r")
                    nc.vector.tensor_scalar_max(rb[:, 0:bsz, :], po[:, 0:bsz, D:NE], 1.0)
                    nc.vector.reciprocal(rb[:, 0:bsz, :], rb[:, 0:bsz, :])
                    nc.vector.tensor_mul(
                        out_sb[:, b0:b0 + bsz, :],
                        po[:, 0:bsz, 0:D],
                        rb[:, 0:bsz, :].to_broadcast([P, bsz, D]),
                    )
                # y stores ride the Scalar HWDGE ring so they can't
                # head-of-line block the x loads on the Sync ring
                nc.scalar.dma_start(yv[:, tg:tg + nt, :], out_sb[:])

        nc._prefetch_waits = {}

    return nc


_cached_nc = None


def _get_nc():
    global _cached_nc
    if _cached_nc is None:
        _cached_nc = build_kernel()
    return _cached_nc


def _blocked_T(src):
    """[n, R, V] -> [n, V, R] contiguous, cache-blocked (3x faster than
    numpy's strided transpose copy on 100MB inputs)."""
    n, R, Vd = src.shape
    out = np.empty((n, Vd, R), src.dtype)
    Bk = 128
    for k in range(n):
        s, o = src[k], out[k]
        for i in range(0, R, Bk):
            for j in range(0, Vd, Bk):
                o[j:j + Bk, i:i + Bk] = s[i:i + Bk, j:j + Bk].T
    return out


def make_in_maps(inputs):
    """Host-side shard prep: batch-shard x vocab-major bf16 (x is 0/1 ->
    exact); pack [E | 1] bf16 partition-major."""
    import ml_dtypes

    bf16 = ml_dtypes.bfloat16
    fp8 = ml_dtypes.float8_e4m3fn
    x = np.asarray(inputs["batch_vectors"], dtype=np.float32).reshape(NCORES, ROWS, V)
    e = np.asarray(inputs["embedding_matrix"], dtype=np.float32)
    e_aug = np.concatenate([e, np.ones((V, 1), dtype=np.float32)], axis=1)
    e_dev = np.ascontiguousarray(
        e_aug.reshape(NCH, P, NE).transpose(1, 0, 2).astype(bf16)
    )
    xt = _blocked_T(x.astype(fp8))  # [8, 1024, 3200] fp8 (0/1 -> exact)
    return [{"x": xt[i], "emb": e_dev} for i in range(NCORES)]


_ldw_patched = False


def _enable_ldw_opt():
    """walrus ships with --enable-ldw-opt=false; enabling it lets codegen
    overlap LDWEIGHTS with the previous matmul's streaming, which is worth
    ~2x on our ldweights-heavy matmul cadence."""
    global _ldw_patched
    if _ldw_patched:
        return
    import concourse.bass_utils as bu

    orig = bu.run_command

    def run_command_ldw(cmd, *a, **kw):
        if isinstance(cmd, list):
            cmd = ["--enable-ldw-opt=true" if c == "--enable-ldw-opt=false" else c
                   for c in cmd]
        return orig(cmd, *a, **kw)

    bu.run_command = run_command_ldw
    _ldw_patched = True


def kernel(**inputs):
    from concourse.bass_utils import run_bass_kernel_spmd


    in_maps = make_in_maps(inputs)
    res = run_bass_kernel_spmd(_get_nc(), in_maps, core_ids=list(range(NCORES)))
    out = np.concatenate(
        [res.results[i]["y"].reshape(PER_CORE_B, S, D) for i in range(NCORES)],
        axis=0,
    )
    return out.astype(np.float32)


# revision 21
# speedup vs baseline: 2.2088x; 1.0949x over previous
"""Trainium2 Bass kernel for nn_KC_Avg_Embedding (multi-hot averaged embedding).

Computes, for multi-hot indicator vectors x[b,s,:] over a vocabulary of 1024:
    out[b,s,:] = (x[b,s,:] @ E) / max(sum(x[b,s,:]), 1)

Strategy (data-parallel over 8 NeuronCores, batch-sharded):
  - Each core gets rows = (B/8)*S = 3200 rows of x plus the full embedding
    matrix E [1024, 128].
  - The shard is staged vocab-major bf16 on the host (x is 0/1 so bf16 is
    exact) so the device DMA lands it with the vocab dim on partitions — no
    on-chip transposes, no casts, no PSUM->SBUF copybacks, and every DMA can
    use the fast HWDGE path.
  - E is host-rounded to bf16 (rel err ~2e-3, inside the 2e-2 gate) and
    host-packed to [128 part, 8 chunk, 129] with a ones column appended so
    each matmul also accumulates the row count.
  - E and the first x groups are issued as raw DMAs *before* the TileContext
    so the HBM stream runs under the fixed ~6us engine-boot/preamble cost;
    their consumers get semaphore waits attached at commit time (attaching to
    the instruction is what survives the tile scheduler's reordering).
  - x loads ride the Sync HWDGE ring, y stores the Scalar HWDGE ring —
    separate FIFOs, so a compute-blocked store can't head-of-line block the
    x stream.
  - Per 128-row tile: 8 accumulating matmuls (stationary xT chunk, moving
    [E | 1]) -> PSUM [128 rows, 129] = [x@E | count]; DVE epilogue computes
    1/max(count,1) and scales.
"""

import sys
from contextlib import ExitStack

import numpy as np

for _p in ("/opt/trn_rl_repo",):
    if _p not in sys.path:
        sys.path.insert(0, _p)

import concourse.bass as bass
import concourse.mybir as mybir
import concourse.tile as tile

from concourse.vector_clock import ScopedClock


class _SplitDrainTC(tile.TileContext):
    """TileContext tweaks for this walrus build:

    - attaches prefetch-DMA semaphore waits to the first committed instruction
      that references each prefetched tensor (standalone EventSemaphore waits
      get reordered past their consumers by the tile scheduler);
    - splits multi-semaphore waits across single-wait carrier nops (this
      walrus enforces a one-sync-wait-per-instruction codegen limit)."""

    def _commit_instruction(self, inst, lazy_reg_writes: bool = True):
        pw = getattr(self.nc, "_prefetch_waits", None)
        if pw and getattr(inst, "ins", None):
            for arg in list(inst.ins):
                t = getattr(arg, "tensor", None)
                if t is None:
                    bap = getattr(arg, "bass_ap", None)
                    t = getattr(bap, "tensor", None) if bap is not None else None
                nm = getattr(t, "name", None)
                if nm in pw:
                    sem, val = pw.pop(nm)
                    bass.BassInstruction(inst).wait_op(sem, val, "sem-ge")
        si = getattr(inst, "sync_info", None)
        if (
            si is not None
            and si.on_wait
            and len(si.on_wait) > 1
            and inst.engine != mybir.EngineType.Unassigned
        ):
            waits = list(si.on_wait)
            del si.on_wait[1:]
            for w in waits[1:]:
                nop = mybir.InstNoOp(
                    name=self.nc.get_next_instruction_name(),
                    engine=inst.engine,
                    sync_info=mybir.SyncInfo(on_wait=[w], on_update=[]),
                    bass_nofuse=True,
                )
                super()._commit_instruction(nop, lazy_reg_writes)
        super()._commit_instruction(inst, lazy_reg_writes)

    def _drain_and_barrier(self, tick_clock, wait_clock):
        drain_inst = self.nc.sync.drain()
        wait_clock.add_sem_waits(
            drain_inst.ins, ScopedClock({None: tick_clock.global_clock})
        )
        si = drain_inst.ins.sync_info
        if si is not None and si.on_wait is not None and len(si.on_wait) > 1:
            waits = list(si.on_wait)
            del si.on_wait[1:]
            for w in waits[1:]:
                nop = self.nc.sync.nop(nofuse=True, hint="drain_wait_split")
                nsi = nop.ins.sync_info
                if nsi is None:
                    nop.ins.sync_info = mybir.SyncInfo(on_update=[], on_wait=[w])
                else:
                    nsi.on_wait.append(w)
        self.nc.all_engine_barrier()
        assert self.sems is not None
        popped = self.nc._tile_sem_poison_stack.pop()
        assert popped is self._sem_poison
        self.nc.clear_and_free_semaphores(list(self.sems.allocated().values()))
        self.nc.all_engine_barrier()


B, S, V, D = 128, 200, 1024, 128
NCORES = 8
P = 128
PER_CORE_B = B // NCORES          # 16
ROWS = PER_CORE_B * S             # 3200 rows per core
NT = ROWS // P                    # 25 row tiles per core
NCH = V // P                      # 8 vocab chunks
NE = D + 1                        # 128 emb cols + 1 count col
# row tiles per DMA group: ramp up for fast first compute, ramp down so the
# last x bytes arrive when little work remains
GROUPS = (2, 3, 4, 5, 5, 4, 2)
N_PREFETCH = 3                    # x groups issued as raw DMAs before tc entry
assert sum(GROUPS) == NT
# byte offset (in elements) of each group in the host-packed x
_OFFS = []
_o = 0
for _nt in GROUPS:
    _OFFS.append(_o)
    _o += _nt * P * NCH


def build_kernel():
    nc = bass.Bass()
    # x arrives host-packed fp8, per-partition contiguous per group:
    # x[p, off_g + c*(nt*128) + r] = xT[c*128+p, tg*128+r] — so each group DMA
    # is one contiguous multi-KB segment per partition (fp8 made the
    # vocab-major segments only 128-640B, well under DMA line rate)
    x = nc.declare_dram_parameter(
        "x", [P, NT * NCH * P], mybir.dt.float8e4, isOutput=False
    )
    # emb arrives host-packed bf16: [128 part, 8 chunk, 129] with ones column
    emb = nc.declare_dram_parameter(
        "emb", [P, NCH, NE], mybir.dt.bfloat16, isOutput=False
    )
    # y is stored bf16 (adds ~2e-3 rel err on top of bf16-E's ~1.7e-3; gate is
    # 2e-2) and widened to fp32 on the host — halves the store traffic.
    y = nc.declare_dram_parameter("y", [ROWS, D], mybir.dt.bfloat16, isOutput=True)

    bf16 = mybir.dt.bfloat16
    fp8 = mybir.dt.float8e4
    f32 = mybir.dt.float32

    yv = y.rearrange("(t p) d -> p t d", p=P)     # [128, 25, 128]

    bounds = []
    t0 = 0
    for nt in GROUPS:
        bounds.append((t0, nt))
        t0 += nt

    with ExitStack() as ctx:
        # Raw SBUF tensors + prefetch DMAs, emitted before the TileContext so
        # the HBM stream runs under the entry preamble. HWDGE (sync ring).
        rhs_t = ctx.enter_context(nc.sbuf_tensor([P, NCH, NE], bf16))
        rhs = rhs_t[:, :, :]
        pre_x = []
        for g in range(N_PREFETCH):
            tg, nt = bounds[g]
            h = ctx.enter_context(nc.sbuf_tensor([P, NCH, nt * P], fp8))
            pre_x.append(h[:, :, :])

        sems = [nc.alloc_semaphore(f"pref{i}") for i in range(1 + N_PREFETCH)]
        nc._prefetch_waits = {}
        for s in sems:
            nc.sync.sem_clear(s)
        nc.sync.dma_start(rhs, emb[:, :, :]).then_inc(sems[0], 16)
        nc._prefetch_waits[rhs_t.name] = (sems[0], 16)
        for g in range(N_PREFETCH):
            tg, nt = bounds[g]
            nc.sync.dma_start(
                pre_x[g].rearrange("p c r -> p (c r)"),
                x[:, _OFFS[g]:_OFFS[g] + nt * P * NCH],
            ).then_inc(sems[1 + g], 16)
            nc._prefetch_waits[pre_x[g].tensor.name] = (sems[1 + g], 16)

        with _SplitDrainTC(nc) as tc, ExitStack() as pools:
            xb_pool = pools.enter_context(
                tc.tile_pool(name="xb", bufs=len(GROUPS) - N_PREFETCH)
            )
            out_pool = pools.enter_context(tc.tile_pool(name="out", bufs=len(GROUPS)))
            small = pools.enter_context(tc.tile_pool(name="small", bufs=12))
            psum_o = pools.enter_context(
                tc.tile_pool(name="psum_o", bufs=8, space="PSUM")
            )

            for g, (tg, nt) in enumerate(bounds):
                if g < N_PREFETCH:
                    xb = pre_x[g]
                else:
                    xb = xb_pool.tile([P, NCH, nt * P], fp8, name=f"xb{tg}", tag="xb")
                    nc.sync.dma_start(
                        xb[:].rearrange("p c r -> p (c r)"),
                        x[:, _OFFS[g]:_OFFS[g] + nt * P * NCH],
                    )
                out_sb = out_pool.tile([P, nt, D], bf16, name=f"out{tg}", tag="out")
                # Batch up to 3 row tiles per PSUM bank (3x516B <= 2KB) so the
                # epilogue is one max+recip+broadcast-mul per batch instead of
                # per tile — a per-tile epilogue saturated DVE/ACT and
                # backpressured the matmul pipeline through PSUM-slot reuse.
                for b0 in range(0, nt, 3):
                    bsz = min(3, nt - b0)
                    po = psum_o.tile([P, 3, NE], f32, name=f"po{tg}_{b0}", tag="po")
                    for j in range(bsz):
                        f = b0 + j
                        for c in range(NCH):
                            nc.tensor.matmul(po[:, j, :],
                                             xb[:, c, f * P:(f + 1) * P],
                                             rhs[:, c, :],
                                             start=(c == 0), stop=(c == NCH - 1))
                    rb = small.tile([P, 3, 1], f32, name=f"r{tg}_{b0}", tag="r")
                    nc.vector.tensor_scalar_max(rb[:, 0:bsz, :], po[:, 0:bsz, D:NE], 1.0)
                    nc.vector.reciprocal(rb[:, 0:bsz, :], rb[:, 0:bsz, :])
                    nc.vector.tensor_mul(
                        out_sb[:, b0:b0 + bsz, :],
                        po[:, 0:bsz, 0:D],
                        rb[:, 0:bsz, :].to_broadcast([P, bsz, D]),
                    )
                # y stores ride the Scalar HWDGE ring so they can't
                # head-of-line block the x loads on the Sync ring
                nc.scalar.dma_start(yv[:, tg:tg + nt, :], out_sb[:])

        nc._prefetch_waits = {}

    return nc


_cached_nc = None


def _get_nc():
    global _cached_nc
    if _cached_nc is None:
        _cached_nc = build_kernel()
    return _cached_nc


def _blocked_T(src):
    """[n, R, V] -> [n, V, R] contiguous, cache-blocked (3x faster than
    numpy's strided transpose copy on 100MB inputs)."""
    n, R, Vd = src.shape
    out = np.empty((n, Vd, R), src.dtype)
    Bk = 128
    for k in range(n):
        s, o = src[k], out[k]
        for i in range(0, R, Bk):
            for j in range(0, Vd, Bk):
                o[j:j + Bk, i:i + Bk] = s[i:i + Bk, j:j + Bk].T
    return out


def make_in_maps(inputs):
    """Host-side shard prep: batch-shard x vocab-major bf16 (x is 0/1 ->
    exact); pack [E | 1] bf16 partition-major."""
    import ml_dtypes

    bf16 = ml_dtypes.bfloat16
    fp8 = ml_dtypes.float8_e4m3fn
    x = np.asarray(inputs["batch_vectors"], dtype=np.float32).reshape(NCORES, ROWS, V)
    e = np.asarray(inputs["embedding_matrix"], dtype=np.float32)
    e_aug = np.concatenate([e, np.ones((V, 1), dtype=np.float32)], axis=1)
    e_dev = np.ascontiguousarray(
        e_aug.reshape(NCH, P, NE).transpose(1, 0, 2).astype(bf16)
    )
    xt = _blocked_T(x.astype(fp8))  # [8, 1024, 3200] fp8 (0/1 -> exact)
    # pack per-partition contiguous per group: [8, 128, sum(nt*8*128)]
    parts = []
    for (tg, nt), off in zip(
        [(sum(GROUPS[:i]), GROUPS[i]) for i in range(len(GROUPS))], _OFFS
    ):
        blk = xt[:, :, tg * P:(tg + nt) * P]          # [8, 1024, nt*128]
        parts.append(
            np.ascontiguousarray(
                blk.reshape(NCORES, NCH, P, nt * P).transpose(0, 2, 1, 3)
            ).reshape(NCORES, P, nt * P * NCH)
        )
    xpack = np.concatenate(parts, axis=2)
    return [{"x": xpack[i], "emb": e_dev} for i in range(NCORES)]


_ldw_patched = False


def _enable_ldw_opt():
    """walrus ships with --enable-ldw-opt=false; enabling it lets codegen
    overlap LDWEIGHTS with the previous matmul's streaming, which is worth
    ~2x on our ldweights-heavy matmul cadence."""
    global _ldw_patched
    if _ldw_patched:
        return
    import concourse.bass_utils as bu

    orig = bu.run_command

    def run_command_ldw(cmd, *a, **kw):
        if isinstance(cmd, list):
            cmd = ["--enable-ldw-opt=true" if c == "--enable-ldw-opt=false" else c
                   for c in cmd]
        return orig(cmd, *a, **kw)

    bu.run_command = run_command_ldw
    _ldw_patched = True


def kernel(**inputs):
    from concourse.bass_utils import run_bass_kernel_spmd


    in_maps = make_in_maps(inputs)
    res = run_bass_kernel_spmd(_get_nc(), in_maps, core_ids=list(range(NCORES)))
    out = np.concatenate(
        [res.results[i]["y"].reshape(PER_CORE_B, S, D) for i in range(NCORES)],
        axis=0,
    )
    return out.astype(np.float32)
